# revision 11
# baseline (speedup 1.0000x reference)
"""AdaptiveGridMerger Trainium2 kernel.

Math: the reference scatters x[b,c,:] into a flat 8x8 grid with bilinear
(4-corner) weights from positions[b,c,:], then matmuls grid_weights.
Equivalent form used here: out[b] = GW @ (S[b] @ x[b]) where
S[b] in R[64,306] holds channel c's 4 corner weights in column c.
S[b].T (layout [c, g]) is built on-device: floor/weights on the vector
engines, then one broadcast outer-product wy (x) wx per row (the 8x8
grid factorizes). Both contractions run on the TensorEngine in bf16.

Sharding: data-parallel over batch, 2 batches per core, grid_weights
replicated (pre-transposed on host to [64, 270] for the lhsT layout).
"""

import numpy as np

import concourse.bass as bass
import concourse.bacc as bacc
import concourse.mybir as mybir
from concourse import tile
from concourse.bass_utils import run_bass_kernel_spmd

B, C, T = 16, 306, 4096
M, G, GS = 270, 64, 8
N_CORES = 8
BL = B // N_CORES  # batches per core

C_CHUNKS = [(0, 128), (128, 128), (256, 50)]
M_CHUNKS = [(0, 128), (128, 128), (256, 14)]
T_DMA = 2048
T_PS = 512

MM_DTYPE = mybir.dt.bfloat16
NP_MM = mybir.dt.np(MM_DTYPE)

FP32 = mybir.dt.float32
OP = mybir.AluOpType


def build_nc():
    nc = bacc.Bacc()
    x_ext = nc.declare_dram_parameter("x", [BL, C, T], MM_DTYPE, isOutput=False)
    pos_ext = nc.declare_dram_parameter("positions", [BL, C, 2], FP32, isOutput=False)
    gwt_ext = nc.declare_dram_parameter("gw_t", [G, M], MM_DTYPE, isOutput=False)
    out_ext = nc.declare_dram_parameter("out", [BL, M, T], FP32, isOutput=True)

    with tile.TileContext(nc) as tc:
        with (
            tc.tile_pool(name="const", bufs=1) as constp,
            tc.tile_pool(name="stp", bufs=1) as stp,
            tc.tile_pool(name="scr", bufs=6) as scr,
            tc.tile_pool(name="xp", bufs=2) as xp,
            tc.tile_pool(name="op", bufs=2) as outp,
            tc.tile_pool(name="gvp", bufs=3) as gvp,
            tc.tile_pool(name="ps_gv", bufs=2, space=bass.MemorySpace.PSUM) as ps_gv,
            tc.tile_pool(name="ps_out", bufs=4, space=bass.MemorySpace.PSUM) as ps_out,
        ):
            gw_t = constp.tile([G, M], MM_DTYPE, tag="gw_t")
            nc.sync.dma_start(out=gw_t[:], in_=gwt_ext[:])

            # iota rows [0..7] and [-1..6]: (iota0 == low) selects the low
            # cell, (iotam1 == low) the low+1 cell. gpsimd generates; a DVE
            # copy gives the DVE its own same-engine-produced pair.
            io_g, io_v = {}, {}
            for nm, base in (("i0", 0), ("im1", -1)):
                tg = constp.tile([128, GS], FP32, tag=f"iog{nm}", name=f"iog{nm}")
                nc.gpsimd.iota(
                    tg[:],
                    pattern=[[1, GS]],
                    base=base,
                    channel_multiplier=0,
                    allow_small_or_imprecise_dtypes=True,
                )
                tv = constp.tile([128, GS], FP32, tag=f"iov{nm}", name=f"iov{nm}")
                nc.vector.tensor_copy(tv[:], tg[:])
                io_g[nm], io_v[nm] = tg, tv

            # Build ST[c, g] = wy[c, gy] * wx[c, gx] (g = gy*8 + gx), where
            # wy/wx hold (1-frac) at floor(pos) and frac at floor(pos)+1.
            # Batch 0 on DVE, batch 1 on GpSimd to halve the setup latency.
            st_tiles = {}
            for b in range(BL):
                eng = nc.vector
                iot = io_v
                for ci, (c0, cn) in enumerate(C_CHUNKS):
                    sfx = f"{b}_{ci}"
                    pos_t = scr.tile([128, 2], FP32, tag=f"pos{sfx}", name=f"pos{sfx}")
                    nc.sync.dma_start(out=pos_t[:cn, :], in_=pos_ext[b, c0 : c0 + cn, :])
                    # grid_pos = (pos + 1) * 4, exact vs reference's *8/2
                    gp = scr.tile([128, 2], FP32, tag=f"gp{sfx}", name=f"gp{sfx}")
                    eng.tensor_scalar(gp[:cn], pos_t[:cn], 1.0, GS / 2.0, OP.add, OP.mult)
                    # floor(): int cast, then subtract 1 where the cast rounded up
                    ilow = scr.tile([128, 2], mybir.dt.int32, tag=f"il{sfx}", name=f"il{sfx}")
                    eng.tensor_copy(ilow[:cn], gp[:cn])
                    flow = scr.tile([128, 2], FP32, tag=f"fl{sfx}", name=f"fl{sfx}")
                    eng.tensor_copy(flow[:cn], ilow[:cn])
                    mask = scr.tile([128, 2], FP32, tag=f"mk{sfx}", name=f"mk{sfx}")
                    eng.tensor_tensor(mask[:cn], flow[:cn], gp[:cn], OP.is_gt)
                    low = scr.tile([128, 2], FP32, tag=f"lo{sfx}", name=f"lo{sfx}")
                    eng.tensor_tensor(low[:cn], flow[:cn], mask[:cn], OP.subtract)
                    whi = scr.tile([128, 2], FP32, tag=f"wh{sfx}", name=f"wh{sfx}")
                    eng.tensor_tensor(whi[:cn], gp[:cn], low[:cn], OP.subtract)
                    wlo = scr.tile([128, 2], FP32, tag=f"wl{sfx}", name=f"wl{sfx}")
                    eng.tensor_scalar(wlo[:cn], whi[:cn], -1.0, 1.0, OP.mult, OP.add)

                    wyx = []
                    for d, nm in ((0, "wy"), (1, "wx")):
                        t1 = scr.tile([128, GS], FP32, tag=f"{nm}a{sfx}", name=f"{nm}a{sfx}")
                        eng.tensor_scalar(
                            t1[:cn], iot["i0"][:cn], low[:cn, d : d + 1],
                            wlo[:cn, d : d + 1], OP.is_equal, OP.mult,
                        )
                        t2 = scr.tile([128, GS], FP32, tag=f"{nm}b{sfx}", name=f"{nm}b{sfx}")
                        eng.tensor_scalar(
                            t2[:cn], iot["im1"][:cn], low[:cn, d : d + 1],
                            whi[:cn, d : d + 1], OP.is_equal, OP.mult,
                        )
                        tw = scr.tile([128, GS], FP32, tag=f"{nm}{sfx}", name=f"{nm}{sfx}")
                        eng.tensor_tensor(tw[:cn], t1[:cn], t2[:cn], OP.add)
                        wyx.append(tw)

                    st = stp.tile([128, G], MM_DTYPE, tag=f"st{sfx}", name=f"st{sfx}")
                    eng.tensor_tensor(
                        st[:cn].rearrange("c (i j) -> c i j", i=GS),
                        wyx[0][:cn].unsqueeze(2).broadcast_to((cn, GS, GS)),
                        wyx[1][:cn].unsqueeze(1).broadcast_to((cn, GS, GS)),
                        OP.mult,
                    )
                    st_tiles[(b, ci)] = st

            # Warm-up matmuls: PE observes the DVE/GpSimd/DMA semaphores here
            # so steady-state matmuls carry few pending waits.
            with tc.tile_pool(name="ps_warm", bufs=1, space=bass.MemorySpace.PSUM) as ps_warm:
                warm = ps_warm.tile([128, G], FP32, tag="warm")
                for b in range(BL):
                    for ci, (c0, cn) in enumerate(C_CHUNKS):
                        st = st_tiles[(b, ci)]
                        nc.tensor.matmul(
                            warm[:G], st[:cn], st[:cn, :G], start=True, stop=True
                        )
                nc.tensor.matmul(
                    warm[:, :G], gw_t[:, :128], gw_t[:, :G], start=True, stop=True
                )

            # Main loop: gv = ST.T @ x (per 512-col chunk), out = GW @ gv
            for b in range(BL):
                for tt in range(T // T_DMA):
                    t0 = tt * T_DMA
                    xts = []
                    for ci, (c0, cn) in enumerate(C_CHUNKS):
                        xt = xp.tile([128, T_DMA], MM_DTYPE, tag=f"x{ci}", name=f"x{ci}")
                        nc.sync.dma_start(
                            out=xt[:cn], in_=x_ext[b, c0 : c0 + cn, t0 : t0 + T_DMA]
                        )
                        xts.append(xt)
                    outs = []
                    for mi, (m0, mn) in enumerate(M_CHUNKS):
                        outs.append(
                            outp.tile([128, T_DMA], FP32, tag=f"o{mi}", name=f"o{mi}")
                        )

                    for ps in range(T_DMA // T_PS):
                        f0 = ps * T_PS
                        gv_ps = ps_gv.tile([G, T_PS], FP32, tag="gv", name="gv")
                        for ci, (c0, cn) in enumerate(C_CHUNKS):
                            nc.tensor.matmul(
                                gv_ps[:],
                                st_tiles[(b, ci)][:cn],
                                xts[ci][:cn, f0 : f0 + T_PS],
                                start=(ci == 0),
                                stop=(ci == len(C_CHUNKS) - 1),
                            )
                        gv_sb = gvp.tile([G, T_PS], MM_DTYPE, tag="gv_sb", name="gv_sb")
                        nc.scalar.copy(gv_sb[:], gv_ps[:])
                        for mi, (m0, mn) in enumerate(M_CHUNKS):
                            o_ps = ps_out.tile([128, T_PS], FP32, tag="o_ps", name="o_ps")
                            nc.tensor.matmul(
                                o_ps[:mn],
                                gw_t[:, m0 : m0 + mn],
                                gv_sb[:],
                                start=True,
                                stop=True,
                            )
                            # split PSUM->SBUF copies between DVE and ACT
                            ceng = nc.scalar if (ps % 2 == 1 and mi == 2) or (
                                ps % 2 == 0 and mi == 1
                            ) else nc.vector
                            if ceng is nc.scalar:
                                ceng.copy(outs[mi][:mn, f0 : f0 + T_PS], o_ps[:mn])
                            else:
                                ceng.tensor_copy(outs[mi][:mn, f0 : f0 + T_PS], o_ps[:mn])
                    for mi, (m0, mn) in enumerate(M_CHUNKS):
                        nc.sync.dma_start(
                            out=out_ext[b, m0 : m0 + mn, t0 : t0 + T_DMA],
                            in_=outs[mi][:mn],
                        )
    nc.compile()
    return nc


def make_in_maps(x, positions, grid_weights):
    gw_t = np.ascontiguousarray(grid_weights.T).astype(NP_MM)
    in_maps = []
    for i in range(N_CORES):
        sl = slice(i * BL, (i + 1) * BL)
        in_maps.append(
            {
                "x": np.ascontiguousarray(x[sl]).astype(NP_MM),
                "positions": np.ascontiguousarray(positions[sl]),
                "gw_t": gw_t,
            }
        )
    return in_maps


_NC_CACHE = None


def kernel(x, positions, grid_weights):
    global _NC_CACHE
    if _NC_CACHE is None:
        _NC_CACHE = build_nc()
    nc = _NC_CACHE
    in_maps = make_in_maps(x, positions, grid_weights)
    res = run_bass_kernel_spmd(nc, in_maps, core_ids=list(range(N_CORES)))
    out = np.concatenate([r["out"] for r in res.results], axis=0)
    return out.astype(np.float32)


if __name__ == "__main__":
    xs = np.random.randn(B, C, T).astype(np.float32)
    ps = np.random.uniform(-1, 0.74, (B, C, 2)).astype(np.float32)
    gw = np.random.randn(M, G).astype(np.float32)
    out = kernel(xs, ps, gw)
    print(out.shape, out.dtype)


# revision 14
# speedup vs baseline: 1.3578x; 1.3578x over previous
"""AdaptiveGridMerger Trainium2 kernel.

Math: the reference scatters x[b,c,:] into a flat 8x8 grid with bilinear
(4-corner) weights from positions[b,c,:], then matmuls grid_weights.
Equivalent form used here: out[b] = GW @ (S[b] @ x[b]) where
S[b] in R[64,306] holds channel c's 4 corner weights in column c.
S[b].T (layout [c, g]) is built on-device: floor/weights on the vector
engines, then one broadcast outer-product wy (x) wx per row (the 8x8
grid factorizes). Both contractions run on the TensorEngine in bf16.

Sharding: data-parallel over batch, 2 batches per core, grid_weights
replicated (pre-transposed on host to [64, 270] for the lhsT layout).
"""

import numpy as np

import concourse.bass as bass
import concourse.bacc as bacc
import concourse.mybir as mybir
from concourse import tile
from concourse.bass_utils import run_bass_kernel_spmd

B, C, T = 16, 306, 4096
M, G, GS = 270, 64, 8
N_CORES = 8
BL = B // N_CORES  # batches per core

C_CHUNKS = [(0, 128), (128, 128), (256, 50)]
M_CHUNKS = [(0, 128), (128, 128), (256, 14)]
T_DMA = 2048
T_PS = 512

MM_DTYPE = mybir.dt.bfloat16
NP_MM = mybir.dt.np(MM_DTYPE)

FP32 = mybir.dt.float32
OP = mybir.AluOpType


def build_nc():
    nc = bacc.Bacc()
    x_ext = nc.declare_dram_parameter("x", [BL, C, T], MM_DTYPE, isOutput=False)
    pos_ext = nc.declare_dram_parameter("positions", [BL, C, 2], FP32, isOutput=False)
    gwt_ext = nc.declare_dram_parameter("gw_t", [G, M], MM_DTYPE, isOutput=False)
    out_ext = nc.declare_dram_parameter("out", [BL, M, T], FP32, isOutput=True)

    with tile.TileContext(nc) as tc:
        with (
            tc.tile_pool(name="const", bufs=1) as constp,
            tc.tile_pool(name="stp", bufs=1) as stp,
            tc.tile_pool(name="scr", bufs=6) as scr,
            tc.tile_pool(name="xp", bufs=2) as xp,
            tc.tile_pool(name="op", bufs=2) as outp,
            tc.tile_pool(name="gvp", bufs=3) as gvp,
            tc.tile_pool(name="ps_gv", bufs=1, space=bass.MemorySpace.PSUM) as ps_gv,
            tc.tile_pool(name="ps_out", bufs=4, space=bass.MemorySpace.PSUM) as ps_out,
        ):
            gw_t = constp.tile([G, M], MM_DTYPE, tag="gw_t")
            nc.sync.dma_start(out=gw_t[:], in_=gwt_ext[:])

            # iota rows [0..7] and [-1..6]: (iota0 == low) selects the low
            # cell, (iotam1 == low) the low+1 cell. gpsimd generates; a DVE
            # copy gives the DVE its own same-engine-produced pair.
            io_g, io_v = {}, {}
            for nm, base in (("i0", 0), ("im1", -1)):
                tg = constp.tile([128, GS], FP32, tag=f"iog{nm}", name=f"iog{nm}")
                nc.gpsimd.iota(
                    tg[:],
                    pattern=[[1, GS]],
                    base=base,
                    channel_multiplier=0,
                    allow_small_or_imprecise_dtypes=True,
                )
                tv = constp.tile([128, GS], FP32, tag=f"iov{nm}", name=f"iov{nm}")
                nc.vector.tensor_copy(tv[:], tg[:])
                io_g[nm], io_v[nm] = tg, tv

            # Build ST[c, g] = wy[c, gy] * wx[c, gx] (g = gy*8 + gx), where
            # wy/wx hold (1-frac) at floor(pos) and frac at floor(pos)+1.
            # Batch 0 on DVE, batch 1 on GpSimd to halve the setup latency.
            st_tiles = {}
            for b in range(BL):
                eng = nc.vector
                iot = io_v
                for ci, (c0, cn) in enumerate(C_CHUNKS):
                    sfx = f"{b}_{ci}"
                    pos_t = scr.tile([128, 2], FP32, tag=f"pos{sfx}", name=f"pos{sfx}")
                    nc.sync.dma_start(out=pos_t[:cn, :], in_=pos_ext[b, c0 : c0 + cn, :])
                    # grid_pos = (pos + 1) * 4, exact vs reference's *8/2
                    gp = scr.tile([128, 2], FP32, tag=f"gp{sfx}", name=f"gp{sfx}")
                    eng.tensor_scalar(gp[:cn], pos_t[:cn], 1.0, GS / 2.0, OP.add, OP.mult)
                    # floor(): int cast, then subtract 1 where the cast rounded up
                    ilow = scr.tile([128, 2], mybir.dt.int32, tag=f"il{sfx}", name=f"il{sfx}")
                    eng.tensor_copy(ilow[:cn], gp[:cn])
                    flow = scr.tile([128, 2], FP32, tag=f"fl{sfx}", name=f"fl{sfx}")
                    eng.tensor_copy(flow[:cn], ilow[:cn])
                    mask = scr.tile([128, 2], FP32, tag=f"mk{sfx}", name=f"mk{sfx}")
                    eng.tensor_tensor(mask[:cn], flow[:cn], gp[:cn], OP.is_gt)
                    low = scr.tile([128, 2], FP32, tag=f"lo{sfx}", name=f"lo{sfx}")
                    eng.tensor_tensor(low[:cn], flow[:cn], mask[:cn], OP.subtract)
                    whi = scr.tile([128, 2], FP32, tag=f"wh{sfx}", name=f"wh{sfx}")
                    eng.tensor_tensor(whi[:cn], gp[:cn], low[:cn], OP.subtract)
                    wlo = scr.tile([128, 2], FP32, tag=f"wl{sfx}", name=f"wl{sfx}")
                    eng.tensor_scalar(wlo[:cn], whi[:cn], -1.0, 1.0, OP.mult, OP.add)

                    wyx = []
                    for d, nm in ((0, "wy"), (1, "wx")):
                        t1 = scr.tile([128, GS], FP32, tag=f"{nm}a{sfx}", name=f"{nm}a{sfx}")
                        eng.tensor_scalar(
                            t1[:cn], iot["i0"][:cn], low[:cn, d : d + 1],
                            wlo[:cn, d : d + 1], OP.is_equal, OP.mult,
                        )
                        t2 = scr.tile([128, GS], FP32, tag=f"{nm}b{sfx}", name=f"{nm}b{sfx}")
                        eng.tensor_scalar(
                            t2[:cn], iot["im1"][:cn], low[:cn, d : d + 1],
                            whi[:cn, d : d + 1], OP.is_equal, OP.mult,
                        )
                        tw = scr.tile([128, GS], FP32, tag=f"{nm}{sfx}", name=f"{nm}{sfx}")
                        eng.tensor_tensor(tw[:cn], t1[:cn], t2[:cn], OP.add)
                        wyx.append(tw)

                    st = stp.tile([128, G], MM_DTYPE, tag=f"st{sfx}", name=f"st{sfx}")
                    eng.tensor_tensor(
                        st[:cn].rearrange("c (i j) -> c i j", i=GS),
                        wyx[0][:cn].unsqueeze(2).broadcast_to((cn, GS, GS)),
                        wyx[1][:cn].unsqueeze(1).broadcast_to((cn, GS, GS)),
                        OP.mult,
                    )
                    st_tiles[(b, ci)] = st

            # Warm-up matmuls: PE observes the DVE/GpSimd/DMA semaphores here
            # so steady-state matmuls carry few pending waits.
            warm = ps_out.tile([128, T_PS], FP32, tag="o_ps", name="warm")
            for b in range(BL):
                for ci, (c0, cn) in enumerate(C_CHUNKS):
                    st = st_tiles[(b, ci)]
                    nc.tensor.matmul(
                        warm[:G, :G], st[:cn], st[:cn, :G], start=True, stop=True
                    )
            nc.tensor.matmul(
                warm[:, :G], gw_t[:, :128], gw_t[:, :G], start=True, stop=True
            )

            # Main loop: gv = ST.T @ x (per 512-col chunk), out = GW @ gv
            for b in range(BL):
                for tt in range(T // T_DMA):
                    t0 = tt * T_DMA
                    xts = []
                    for ci, (c0, cn) in enumerate(C_CHUNKS):
                        xt = xp.tile([128, T_DMA], MM_DTYPE, tag=f"x{ci}", name=f"x{ci}")
                        nc.sync.dma_start(
                            out=xt[:cn], in_=x_ext[b, c0 : c0 + cn, t0 : t0 + T_DMA]
                        )
                        xts.append(xt)
                    outs = []
                    for mi, (m0, mn) in enumerate(M_CHUNKS):
                        outs.append(
                            outp.tile([128, T_DMA], FP32, tag=f"o{mi}", name=f"o{mi}")
                        )

                    # weights-outer ordering: all column chunks stream through
                    # the SAME stationary operand back-to-back, so the PE can
                    # overlap drain/fill and skip redundant weight reloads.
                    NPS = T_DMA // T_PS
                    gvs = [
                        ps_gv.tile([G, T_PS], FP32, tag=f"gv{ps}", name=f"gv{ps}")
                        for ps in range(NPS)
                    ]
                    for ci, (c0, cn) in enumerate(C_CHUNKS):
                        for ps in range(NPS):
                            f0 = ps * T_PS
                            nc.tensor.matmul(
                                gvs[ps][:],
                                st_tiles[(b, ci)][:cn],
                                xts[ci][:cn, f0 : f0 + T_PS],
                                start=(ci == 0),
                                stop=(ci == len(C_CHUNKS) - 1),
                                skip_group_check=True,
                            )
                    gv_sbs = []
                    for ps in range(NPS):
                        gv_sb = gvp.tile(
                            [G, T_PS], MM_DTYPE, tag=f"gv_sb{ps}", name=f"gv_sb{ps}"
                        )
                        nc.scalar.copy(gv_sb[:], gvs[ps][:])
                        gv_sbs.append(gv_sb)
                    for mi, (m0, mn) in enumerate(M_CHUNKS):
                        for ps in range(NPS):
                            f0 = ps * T_PS
                            o_ps = ps_out.tile([128, T_PS], FP32, tag="o_ps", name="o_ps")
                            nc.tensor.matmul(
                                o_ps[:mn],
                                gw_t[:, m0 : m0 + mn],
                                gv_sbs[ps][:],
                                start=True,
                                stop=True,
                            )
                            # split PSUM->SBUF copies between DVE and ACT
                            ceng = nc.scalar if (ps % 2 == 1 and mi == 2) or (
                                ps % 2 == 0 and mi == 1
                            ) else nc.vector
                            if ceng is nc.scalar:
                                ceng.copy(outs[mi][:mn, f0 : f0 + T_PS], o_ps[:mn])
                            else:
                                ceng.tensor_copy(outs[mi][:mn, f0 : f0 + T_PS], o_ps[:mn])
                    for mi, (m0, mn) in enumerate(M_CHUNKS):
                        nc.sync.dma_start(
                            out=out_ext[b, m0 : m0 + mn, t0 : t0 + T_DMA],
                            in_=outs[mi][:mn],
                        )
    nc.compile()
    return nc


def make_in_maps(x, positions, grid_weights):
    gw_t = np.ascontiguousarray(grid_weights.T).astype(NP_MM)
    in_maps = []
    for i in range(N_CORES):
        sl = slice(i * BL, (i + 1) * BL)
        in_maps.append(
            {
                "x": np.ascontiguousarray(x[sl]).astype(NP_MM),
                "positions": np.ascontiguousarray(positions[sl]),
                "gw_t": gw_t,
            }
        )
    return in_maps


_NC_CACHE = None


def kernel(x, positions, grid_weights):
    global _NC_CACHE
    if _NC_CACHE is None:
        _NC_CACHE = build_nc()
    nc = _NC_CACHE
    in_maps = make_in_maps(x, positions, grid_weights)
    res = run_bass_kernel_spmd(nc, in_maps, core_ids=list(range(N_CORES)))
    out = np.concatenate([r["out"] for r in res.results], axis=0)
    return out.astype(np.float32)


if __name__ == "__main__":
    xs = np.random.randn(B, C, T).astype(np.float32)
    ps = np.random.uniform(-1, 0.74, (B, C, 2)).astype(np.float32)
    gw = np.random.randn(M, G).astype(np.float32)
    out = kernel(xs, ps, gw)
    print(out.shape, out.dtype)


# revision 15
# speedup vs baseline: 1.4584x; 1.0741x over previous
"""AdaptiveGridMerger Trainium2 kernel.

Math: the reference scatters x[b,c,:] into a flat 8x8 grid with bilinear
(4-corner) weights from positions[b,c,:], then matmuls grid_weights.
Equivalent form used here: out[b] = GW @ (S[b] @ x[b]) where
S[b] in R[64,306] holds channel c's 4 corner weights in column c.
S[b].T (layout [c, g]) is built on-device: floor/weights on the vector
engine, then one broadcast outer-product wy (x) wx per row (the 8x8
grid factorizes). Both contractions run on the TensorEngine in bf16.

Sharding: data-parallel over batch, 2 batches per core, grid_weights
replicated (pre-transposed on host to [64, 270] for the lhsT layout).

Perf structure: spin matmuls pre-ramp the PE clock (HAM gate) during
setup; weights-outer matmul ordering overlaps drain/fill; 2-bank PSUM
tiles let PSUM->SBUF copies run 1024 wide, split across DVE and ACT so
the PE never stalls on PSUM reuse.
"""

import numpy as np

import concourse.bass as bass
import concourse.bacc as bacc
import concourse.mybir as mybir
from concourse import tile
from concourse.bass_utils import run_bass_kernel_spmd

B, C, T = 16, 306, 4096
M, G, GS = 270, 64, 8
N_CORES = 8
BL = B // N_CORES  # batches per core

C_CHUNKS = [(0, 128), (128, 128), (256, 50)]
M_CHUNKS = [(0, 128), (128, 128), (256, 14)]
T_DMA = 2048
T_PS = 512
NPS = T_DMA // T_PS
N_SPIN = 12

MM_DTYPE = mybir.dt.bfloat16
NP_MM = mybir.dt.np(MM_DTYPE)

FP32 = mybir.dt.float32
OP = mybir.AluOpType


def build_nc():
    nc = bacc.Bacc()
    x_ext = nc.declare_dram_parameter("x", [BL, C, T], MM_DTYPE, isOutput=False)
    pos_ext = nc.declare_dram_parameter("positions", [BL, C, 2], FP32, isOutput=False)
    gwt_ext = nc.declare_dram_parameter("gw_t", [G, M], MM_DTYPE, isOutput=False)
    out_ext = nc.declare_dram_parameter("out", [BL, M, T], FP32, isOutput=True)

    n_chunks = len(C_CHUNKS)
    with tile.TileContext(nc) as tc:
        with (
            tc.tile_pool(name="const", bufs=1) as constp,
            tc.tile_pool(name="stp", bufs=1) as stp,
            tc.tile_pool(name="scr", bufs=1) as scr,
            tc.tile_pool(name="xp", bufs=3) as xp,
            tc.tile_pool(name="op", bufs=2) as outp,
            tc.tile_pool(name="gvp", bufs=3) as gvp,
            tc.tile_pool(name="ps_gv", bufs=1, space=bass.MemorySpace.PSUM) as ps_gv,
            tc.tile_pool(name="ps_out", bufs=2, space=bass.MemorySpace.PSUM) as ps_out,
        ):
            # ---- PE clock pre-ramp: spin matmuls on a zero tile while the
            # ---- DMAs and the DVE setup below run. HAM releases the 2.4 GHz
            # ---- clock only after ~5 us of sustained PE activity.
            dummy = constp.tile([128, T_PS], MM_DTYPE, tag="dummy")
            nc.gpsimd.memset(dummy[:], 0.0)
            spin_ps = ps_out.tile([128, 2 * T_PS], FP32, tag="o_ps", name="spin_ps")
            for s in range(N_SPIN):
                nc.tensor.matmul(
                    spin_ps[:, :T_PS], dummy[:, :128], dummy[:], start=True, stop=True
                )

            gw_t = constp.tile([G, M], MM_DTYPE, tag="gw_t")
            nc.sync.dma_start(out=gw_t[:], in_=gwt_ext[:])

            # iota rows [0..7] and [-1..6]: (iota0 == low) selects the low
            # cell, (iotam1 == low) the low+1 cell.
            iot = {}
            for nm, base in (("i0", 0), ("im1", -1)):
                tg = constp.tile([128, GS], FP32, tag=f"iog{nm}", name=f"iog{nm}")
                nc.gpsimd.iota(
                    tg[:],
                    pattern=[[1, GS]],
                    base=base,
                    channel_multiplier=0,
                    allow_small_or_imprecise_dtypes=True,
                )
                tv = constp.tile([128, GS], FP32, tag=f"iov{nm}", name=f"iov{nm}")
                nc.vector.tensor_copy(tv[:], tg[:])
                iot[nm] = tv

            # ---- ST build. All 6 (batch, chunk) column-pairs share one
            # ---- [128, 12] tile so the floor/weight math is 7 wide ops.
            NCOL = 2 * BL * n_chunks
            pos_all = scr.tile([128, NCOL], FP32, tag="pos_all")
            nc.vector.memset(pos_all[:], 0.0)
            for b in range(BL):
                for ci, (c0, cn) in enumerate(C_CHUNKS):
                    col = 2 * (b * n_chunks + ci)
                    nc.sync.dma_start(
                        out=pos_all[:cn, col : col + 2],
                        in_=pos_ext[b, c0 : c0 + cn, :],
                    )
            # grid_pos = (pos + 1) * 4, exact vs reference's *8/2
            gp = scr.tile([128, NCOL], FP32, tag="gp")
            nc.vector.tensor_scalar(gp[:], pos_all[:], 1.0, GS / 2.0, OP.add, OP.mult)
            # floor(): int cast, then subtract 1 where the cast rounded up
            ilow = scr.tile([128, NCOL], mybir.dt.int32, tag="ilow")
            nc.vector.tensor_copy(ilow[:], gp[:])
            flow = scr.tile([128, NCOL], FP32, tag="flow")
            nc.vector.tensor_copy(flow[:], ilow[:])
            mask = scr.tile([128, NCOL], FP32, tag="mask")
            nc.vector.tensor_tensor(mask[:], flow[:], gp[:], OP.is_gt)
            low = scr.tile([128, NCOL], FP32, tag="low")
            nc.vector.tensor_tensor(low[:], flow[:], mask[:], OP.subtract)
            whi = scr.tile([128, NCOL], FP32, tag="whi")
            nc.vector.tensor_tensor(whi[:], gp[:], low[:], OP.subtract)
            wlo = scr.tile([128, NCOL], FP32, tag="wlo")
            nc.vector.tensor_scalar(wlo[:], whi[:], -1.0, 1.0, OP.mult, OP.add)

            st_tiles = {}
            for b in range(BL):
                for ci, (c0, cn) in enumerate(C_CHUNKS):
                    sfx = f"{b}_{ci}"
                    col = 2 * (b * n_chunks + ci)
                    wyx = []
                    for d, nm in ((0, "wy"), (1, "wx")):
                        cd = col + d
                        t1 = scr.tile([128, GS], FP32, tag=f"{nm}a{sfx}", name=f"{nm}a{sfx}")
                        nc.vector.tensor_scalar(
                            t1[:cn], iot["i0"][:cn], low[:cn, cd : cd + 1],
                            wlo[:cn, cd : cd + 1], OP.is_equal, OP.mult,
                        )
                        t2 = scr.tile([128, GS], FP32, tag=f"{nm}b{sfx}", name=f"{nm}b{sfx}")
                        nc.vector.tensor_scalar(
                            t2[:cn], iot["im1"][:cn], low[:cn, cd : cd + 1],
                            whi[:cn, cd : cd + 1], OP.is_equal, OP.mult,
                        )
                        tw = scr.tile([128, GS], FP32, tag=f"{nm}{sfx}", name=f"{nm}{sfx}")
                        nc.vector.tensor_tensor(tw[:cn], t1[:cn], t2[:cn], OP.add)
                        wyx.append(tw)
                    st = stp.tile([128, G], MM_DTYPE, tag=f"st{sfx}", name=f"st{sfx}")
                    nc.vector.tensor_tensor(
                        st[:cn].rearrange("c (i j) -> c i j", i=GS),
                        wyx[0][:cn].unsqueeze(2).broadcast_to((cn, GS, GS)),
                        wyx[1][:cn].unsqueeze(1).broadcast_to((cn, GS, GS)),
                        OP.mult,
                    )
                    st_tiles[(b, ci)] = st

            # Warm-up matmuls: PE observes the DVE/DMA semaphores here so
            # steady-state matmuls carry few pending waits.
            warm = ps_out.tile([128, 2 * T_PS], FP32, tag="o_ps", name="warm")
            for b in range(BL):
                for ci, (c0, cn) in enumerate(C_CHUNKS):
                    st = st_tiles[(b, ci)]
                    nc.tensor.matmul(
                        warm[:G, :G], st[:cn], st[:cn, :G], start=True, stop=True
                    )
            nc.tensor.matmul(
                warm[:, :G], gw_t[:, :128], gw_t[:, :G], start=True, stop=True
            )

            # ---- Main loop: gv = ST.T @ x, out = GW @ gv, weights-outer.
            for b in range(BL):
                for tt in range(T // T_DMA):
                    t0 = tt * T_DMA
                    xts = []
                    for ci, (c0, cn) in enumerate(C_CHUNKS):
                        xt = xp.tile([128, T_DMA], MM_DTYPE, tag=f"x{ci}", name=f"x{ci}")
                        nc.sync.dma_start(
                            out=xt[:cn], in_=x_ext[b, c0 : c0 + cn, t0 : t0 + T_DMA]
                        )
                        xts.append(xt)
                    outs = []
                    for mi, (m0, mn) in enumerate(M_CHUNKS):
                        outs.append(
                            outp.tile([128, T_DMA], FP32, tag=f"o{mi}", name=f"o{mi}")
                        )

                    # mm1: 2-bank psum tiles [64, 1024]; ci-outer for weight reuse
                    gvs = [
                        ps_gv.tile([G, 2 * T_PS], FP32, tag=f"gv{h}", name=f"gv{h}")
                        for h in range(NPS // 2)
                    ]
                    for ci, (c0, cn) in enumerate(C_CHUNKS):
                        for ps in range(NPS):
                            f0 = ps * T_PS
                            nc.tensor.matmul(
                                gvs[ps // 2][:, (ps % 2) * T_PS : (ps % 2 + 1) * T_PS],
                                st_tiles[(b, ci)][:cn],
                                xts[ci][:cn, f0 : f0 + T_PS],
                                start=(ci == 0),
                                stop=(ci == n_chunks - 1),
                                skip_group_check=True,
                            )
                    gv_sbs = []
                    for h in range(NPS // 2):
                        gv_sb = gvp.tile(
                            [G, 2 * T_PS], MM_DTYPE, tag=f"gv_sb{h}", name=f"gv_sb{h}"
                        )
                        nc.scalar.copy(gv_sb[:], gvs[h][:])
                        gv_sbs.append(gv_sb)

                    # mm2: mi-outer; [128, 1024] psum; copies 1024 wide
                    for mi, (m0, mn) in enumerate(M_CHUNKS):
                        for h in range(NPS // 2):
                            o_ps = ps_out.tile(
                                [128, 2 * T_PS], FP32, tag="o_ps", name="o_ps"
                            )
                            for q in range(2):
                                nc.tensor.matmul(
                                    o_ps[:mn, q * T_PS : (q + 1) * T_PS],
                                    gw_t[:, m0 : m0 + mn],
                                    gv_sbs[h][:, q * T_PS : (q + 1) * T_PS],
                                    start=True,
                                    stop=True,
                                )
                            f0 = h * 2 * T_PS
                            dst = outs[mi][:mn, f0 : f0 + 2 * T_PS]
                            if mi == 2:
                                nc.scalar.copy(dst, o_ps[:mn])
                            else:
                                nc.vector.tensor_copy(dst, o_ps[:mn])
                    for mi, (m0, mn) in enumerate(M_CHUNKS):
                        nc.sync.dma_start(
                            out=out_ext[b, m0 : m0 + mn, t0 : t0 + T_DMA],
                            in_=outs[mi][:mn],
                        )
    nc.compile()
    return nc


def make_in_maps(x, positions, grid_weights):
    gw_t = np.ascontiguousarray(grid_weights.T).astype(NP_MM)
    in_maps = []
    for i in range(N_CORES):
        sl = slice(i * BL, (i + 1) * BL)
        in_maps.append(
            {
                "x": np.ascontiguousarray(x[sl]).astype(NP_MM),
                "positions": np.ascontiguousarray(positions[sl]),
                "gw_t": gw_t,
            }
        )
    return in_maps


_NC_CACHE = None


def kernel(x, positions, grid_weights):
    global _NC_CACHE
    if _NC_CACHE is None:
        _NC_CACHE = build_nc()
    nc = _NC_CACHE
    in_maps = make_in_maps(x, positions, grid_weights)
    res = run_bass_kernel_spmd(nc, in_maps, core_ids=list(range(N_CORES)))
    out = np.concatenate([r["out"] for r in res.results], axis=0)
    return out.astype(np.float32)


if __name__ == "__main__":
    xs = np.random.randn(B, C, T).astype(np.float32)
    ps = np.random.uniform(-1, 0.74, (B, C, 2)).astype(np.float32)
    gw = np.random.randn(M, G).astype(np.float32)
    out = kernel(xs, ps, gw)
    print(out.shape, out.dtype)
